# revision 53
# baseline (speedup 1.0000x reference)
"""Trainium2 Bass kernel for windowed sparse attention (nn_Attention_17703855194428).

Reference computation (per window w of 128 = B*X*Y, tokens N=294 = L*W1*W2):
    qkv = x_w @ w_qkv.T ; q,k,v heads (8 heads x 32 dim), q scaled by 1/sqrt(32)
    sim = q @ k.T + rel_pos_bias ; masked cols -> -1e9 ; softmax over keys
    out = (attn @ v) @ w_out.T

Sharding: pure data parallel over the 128 independent windows -> 16 windows
per NeuronCore, weights/bias replicated. No collectives.

Kernel layout strategy (per core, per window; all matmul inputs fp16,
fp32 PSUM accumulate; host pre-computes exp(rel-pos-bias) transposed and the
additive mask in the kernel's SBUF layouts):
    xT [C=256, N=294] (channels on partitions) ->
    q,k as [E, N] (head-dim on partitions), v as [N, E] (tokens on partitions)
    simT[j, i] = sum_d k[d,j] q[d,i] via row-tiled (K=32) matmuls, 2 heads
        per 2-bank psum tile, double-buffered (sim_pairs)
    P_T = exp(simT + mask_j) * exp(bias)_T   (one ACT exp with per-partition
        mask bias reading across psum banks + one DVE fp16 multiply)
    outU.T[hd, i] = sum_j v[j, hd] P_T[j, i] via col-tiled (M=32) matmuls
    rowsum broadcast to all 32 head rows for free via col-tiled ones-matmul
    1/rowsum via DVE reciprocal_approx_fast; normalize is one DVE multiply
    out = (outU.T * (1/rowsum)).T @ w_out.T via K=hd matmuls -> [tok, C],
    written fp16 (host upcasts to fp32)

Pipelining (pipe='split2'): QKV emission of window w+1 is split around
the attention of window w (qk ahead of hg0, v between hg0 and hg1), and
the out-projection runs at lag 2 (outproj of w-2 after hg1 of w-1), so
every engine has off-chain work during sim->exp->mul->pv stalls.
I/O batching: one output DMA per window (fo gathered in a [98, 3, 256]
tile, dram viewed as (i p) e -> p i e) and one xt load per window-pair,
prefetched one pair ahead (xt_pre — the pair DMA otherwise stalls the
first q/k matmuls of every even window; -1.4% HW-measured).
Engine placement (HW-measured): all four q/k psum->sbuf copies on ACT
(qk_eng='act'), v copies + eb-mul + recip + normalize on DVE, fo on ACT.

Measured on HW (differential For_i timing): ~248-298 us per core for 16
windows depending on the session (vs ~327-344 us for the session-start
baseline; HW sessions drift +-16 % between processes — only within-
process interleaved A/B comparisons are trustworthy). Engines are nearly
balanced (PE ~12.5 us/win engine-busy, ACT ~11, DVE ~9 in the cost-model
sim); the toolchain serializes every Ldweights+Matmult pair
(--enable-ldw-opt=false), so matmul count matters as much as F-cycles.
Rejected on HW measurement: rowsum-in-PV via augmented V (normalize
broadcast costs exceed the PE win; gpsimd partition_broadcast also reads
physical partition 0, and PV leaves psum rows 33-63 uninitialized),
pool-engine eb-muls (+5% — GPSIMD has no PSUM port and shares the DVE
SBUF port), fp8 DoubleRow (needs cross-partition pair-interleaved
layouts), rs_presum / merged eb-muls / paired out-proj psum banks /
fo_dve / v_act (+3-9% each: chain latency beats instruction count).
"""

import numpy as np
from contextlib import ExitStack

import concourse.bass as bass
import concourse.bacc as bacc
import concourse.mybir as mybir
from concourse import tile
from concourse.bass_utils import run_bass_kernel_spmd

import ml_dtypes

F32 = mybir.dt.float32
F32R = mybir.dt.float32r
BF16 = mybir.dt.bfloat16
FP16 = mybir.dt.float16
EXP = mybir.ActivationFunctionType.Exp

# Problem constants (hardcoded per harness contract)
B, AGENT, X, Y, WIN, DIM, HEADS, DH = 2, 6, 8, 8, 7, 256, 8, 32
N = AGENT * WIN * WIN            # 294 tokens per window
NWIN = B * X * Y                 # 128 windows
NCORES = 8
WPC = NWIN // NCORES             # 16 windows per core
JC = 98                          # key-chunk size (294 = 3*98)
NJC = 3
SCALE = DH ** -0.5
MASK_NEG = -1e9


def _rel_pos_index(L, Wh, Ww):
    coords = np.stack(np.meshgrid(np.arange(L), np.arange(Wh), np.arange(Ww), indexing="ij"))
    cf = coords.reshape(3, -1)
    rel = cf[:, :, None] - cf[:, None, :]
    rel = rel.transpose(1, 2, 0).astype(np.int64)
    rel[..., 0] += L - 1
    rel[..., 1] += Wh - 1
    rel[..., 2] += Ww - 1
    rel[..., 0] *= (2 * Wh - 1) * (2 * Ww - 1)
    rel[..., 1] *= 2 * Ww - 1
    return rel.sum(-1)  # (N, N) [i, j]


def build_graph(n_wins=WPC, wbufs=2, pvrs_bufs=2, misc_bufs=2, trace_sim=False, reps=1,
                v_eng='act', fo_eng='dve', gp_jc=(), sim_pairs=False, timing=False, out16=False, ab_noexp=False, ab_nosim=False, ab_nopv=False, pipe=False, sim_bufs=2, rs_presum=False,
                mul_eng='dve', one_dma=False, xt_pair=False, qk_eng='split',
                augv=False, po_pair=False, mul4=False, xt_pre=False, pv_inline=False):
    nc = bacc.Bacc(None)
    n_xt = 2 if timing else n_wins
    xt_d = nc.declare_dram_parameter("xt", [n_xt, 2, 128, N], FP16, isOutput=False)
    msk_d = nc.declare_dram_parameter("msk", [JC, n_wins * NJC], F32, isOutput=False)
    eb_d = nc.declare_dram_parameter("eb", [JC, HEADS, NJC, N], FP16, isOutput=False)
    wqkv_d = nc.declare_dram_parameter("wqkv", [2, 128, 3 * DIM], FP16, isOutput=False)
    if augv:
        wout_d = nc.declare_dram_parameter("wout", [4, 96, DIM], FP16, isOutput=False)
    else:
        wout_d = nc.declare_dram_parameter("wout", [2, 128, DIM], FP16, isOutput=False)
    ODT = FP16 if out16 else F32
    if timing:
        out_d = nc.dram_tensor("oscr", [n_wins, N, DIM], ODT)
        outx_d = nc.declare_dram_parameter("out", [1, N, DIM], ODT, isOutput=True)
    else:
        out_d = nc.declare_dram_parameter("out", [n_wins, N, DIM], ODT, isOutput=True)
        outx_d = None

    with tile.TileContext(nc, trace_sim=trace_sim) as tc, ExitStack() as ctx:
        cpool = ctx.enter_context(tc.tile_pool(name="consts", bufs=1))
        wpool = ctx.enter_context(tc.tile_pool(name="work", bufs=wbufs))
        # one xt slot per window: slot reuse on DMA-written tiles piles up
        # sync waits beyond what DMA descriptors support
        xpool = ctx.enter_context(tc.tile_pool(name="xin", bufs=n_wins))
        if sim_pairs:
            psim = ctx.enter_context(tc.tile_pool(name="psim", bufs=sim_bufs, space="PSUM"))
        else:
            psim = ctx.enter_context(tc.tile_pool(name="psim", bufs=1, space="PSUM"))
        pvrs = ctx.enter_context(tc.tile_pool(name="pvrs", bufs=pvrs_bufs, space="PSUM"))
        ps1 = ctx.enter_context(tc.tile_pool(name="ps1", bufs=misc_bufs, space="PSUM"))

        # ---- replicated constants ----
        wqkv_sb = []
        for c in range(2):
            t = cpool.tile([128, 3 * DIM], FP16, tag=f"wqkv{c}")
            nc.sync.dma_start(t[:], wqkv_d[c])
            wqkv_sb.append(t)
        wout_sb = []
        for c in range(4 if augv else 2):
            t = cpool.tile([96, DIM] if augv else [128, DIM], FP16,
                           tag=f"wout{c}", name=f"wout{c}")
            nc.sync.dma_start(t[:], wout_d[c])
            wout_sb.append(t)
        eb_sb = cpool.tile([JC, HEADS, NJC, N], FP16, tag="eb")
        nc.sync.dma_start(eb_sb[:], eb_d[:])
        msk_sb = cpool.tile([JC, n_wins * NJC], F32, tag="msk")
        nc.sync.dma_start(msk_sb[:], msk_d[:])
        ones_sb = cpool.tile([JC, 32], FP16, tag="ones")
        nc.vector.memset(ones_sb[:], 1.0)

        if ab_noexp or ab_nosim:
            pt_const = cpool.tile([JC, 4, N], FP16, tag="ptc")
            nc.vector.memset(pt_const[:], 0.01)
            stub = cpool.tile([1, 8], F32, tag="stub")

        # warm-up touches: absorb the one-time const-DMA waits into throwaway
        # instructions so steady-state ops stay within the per-instruction
        # sync-wait budget
        scr_a = cpool.tile([JC, 1], F32, tag="scr_a")
        nc.scalar.copy(scr_a[:], msk_sb[:, 0:1])
        scr_v = cpool.tile([JC, 1], FP16, tag="scr_v")
        nc.vector.tensor_copy(scr_v[:], eb_sb[:, 0, 0, 0:1])

        xt_cache = {}

        def load_xt(w, prefetch=False):
            if xt_pair:
                base = w - (w % 2)
                if base not in xt_cache:
                    # keep earlier pairs resident (prefetch runs ahead)
                    keep = max(2, (int(xt_pre) if xt_pre else 0) // 2 + 1)
                    while len(xt_cache) >= keep:
                        del xt_cache[min(xt_cache)]
                    ts = []
                    for c in range(2):
                        t = xpool.tile([128, 2, N], FP16, tag=f"xt{c}", name=f"xtp{c}")
                        hi = min(base + 2, n_wins)
                        src = xt_d[base % n_xt:(base % n_xt) + (hi - base), c]
                        nc.sync.dma_start(
                            t[:, 0:hi - base, :], src.rearrange("w p n -> p w n"))
                        ts.append(t)
                    xt_cache[base] = ts
                if prefetch:
                    return None
                xt_t = [xt_cache[base][c][:, w - base, :] for c in range(2)]
            else:
                if prefetch:
                    return None
                xt_t = []
                for c in range(2):
                    t = xpool.tile([128, N], FP16, tag=f"xt{c}")
                    nc.sync.dma_start(t[:], xt_d[w % n_xt, c])
                    xt_t.append(t)
            return xt_t

        def emit_qk(w):
            xt_t = load_xt(w)
            qk_sb = []
            for p in range(4):
                ps = ps1.tile([128, 512], F32, tag="b1")
                for c in range(2):
                    nc.tensor.matmul(
                        ps[:, 0:N],
                        lhsT=wqkv_sb[c][:, 128 * p:128 * (p + 1)],
                        rhs=xt_t[c][:],
                        start=(c == 0), stop=(c == 1),
                    )
                t = wpool.tile([128, N], FP16, tag=f"qk{p}")
                use_act = (p < 2) if qk_eng == 'split' else (qk_eng == 'act')
                if use_act:
                    nc.scalar.copy(t[:], ps[:, 0:N])
                else:
                    nc.vector.tensor_copy(t[:], ps[:, 0:N])
                qk_sb.append(t)
            return xt_t, qk_sb

        def emit_v(xt_t):
            v_sb = []
            for j in range(NJC):
                ps = ps1.tile([128, 512], F32, tag="b1")
                for c in range(2):
                    nc.tensor.matmul(
                        ps[0:JC, 0:DIM],
                        lhsT=xt_t[c][:, JC * j:JC * (j + 1)],
                        rhs=wqkv_sb[c][:, 2 * DIM:3 * DIM],
                        start=(c == 0), stop=(c == 1),
                    )
                if augv:
                    # per-head 33rd column of ones: rowsum rides the PV matmul
                    t = wpool.tile([JC, HEADS, DH + 1], FP16, tag=f"v{j}",
                                   name=f"v{j}")
                    dst = t[:, :, 0:DH]
                    if v_eng == 'act':
                        nc.scalar.copy(dst, ps[0:JC, 0:DIM])
                    else:
                        nc.vector.tensor_copy(dst, ps[0:JC, 0:DIM])
                    nc.vector.memset(t[:, :, DH:DH + 1], 1.0)
                else:
                    t = wpool.tile([JC, DIM], FP16, tag=f"v{j}", name=f"v{j}")
                    if v_eng == 'act':
                        nc.scalar.copy(t[:], ps[0:JC, 0:DIM])
                    else:
                        nc.vector.tensor_copy(t[:], ps[0:JC, 0:DIM])
                v_sb.append(t)
            return v_sb

        def emit_qkv(w):
            xt_t, qk_sb = emit_qk(w)
            return qk_sb, emit_v(xt_t)

        def emit_attn_parts(w, qk_sb, v_sb):
            def ebmul(k, out_ap, a_ap, b_ap):
                eng = mul_eng if mul_eng != 'mix' else ('pool' if k % 2 else 'dve')
                if eng == 'pool':
                    nc.gpsimd.tensor_mul(out_ap, a_ap, b_ap)
                else:
                    nc.vector.tensor_mul(out_ap, a_ap, b_ap)
            on_sb = []

            def do_hg(hg):
                if not augv:
                    pv = pvrs.tile([128, 512], F32, tag="pvrs", name="pv")
                    rs = pvrs.tile([128, 512], F32, tag="pvrs", name="rs")
                pts = []
                if ab_nosim:
                    pts = [pt_const] * NJC
                elif sim_pairs:
                    for jc in range(NJC):
                        ptj = wpool.tile([JC, 4, N], FP16, tag=f"pt{jc}")
                        et4 = wpool.tile([JC, 4, N], FP16, tag=f"et4{jc}",
                                         name=f"et4{jc}") if mul4 else None
                        for sg in range(2):
                            smp = psim.tile([128, 1024], F32, tag="sim")
                            for i2 in range(2):
                                t4 = 2 * sg + i2
                                nc.tensor.matmul(
                                    smp[0:JC, 512 * i2:512 * i2 + N],
                                    lhsT=qk_sb[2 + hg][32 * t4:32 * (t4 + 1), JC * jc:JC * (jc + 1)],
                                    rhs=qk_sb[hg][32 * t4:32 * (t4 + 1), :],
                                    start=True, stop=True,
                                    tile_position=(32 * t4, 0),
                                )
                            et = (et4[:, 2 * sg:2 * sg + 2, :] if mul4 else
                                  wpool.tile([JC, 2, N], FP16, tag=f"et{jc}{sg}",
                                             name=f"et{jc}{sg}")[:])
                            sim_ap = smp[0:JC, :].rearrange("p (t x) -> p t x", t=2)[:, :, 0:N]
                            nc.scalar.activation(
                                et, sim_ap, EXP,
                                bias=msk_sb[:, NJC * w + jc:NJC * w + jc + 1],
                            )
                            if not mul4:
                                eb_ap = eb_sb[:, 4 * hg + 2 * sg:4 * hg + 2 * sg + 2, jc, :]
                                ebmul(2 * jc + sg, ptj[:, 2 * sg:2 * sg + 2, :], et, eb_ap)
                        if mul4:
                            eb_ap = eb_sb[:, 4 * hg:4 * (hg + 1), jc, :]
                            ebmul(jc, ptj[:], et4[:], eb_ap)
                        pts.append(ptj)
                        if pv_inline and not (rs_presum or ab_nopv):
                            for t4 in range(4):
                                h = 4 * hg + t4
                                nc.tensor.matmul(
                                    pv[32 * t4:32 * (t4 + 1), 0:N],
                                    lhsT=v_sb[jc][:, 32 * h:32 * (h + 1)],
                                    rhs=ptj[:, t4, :],
                                    start=(jc == 0), stop=(jc == NJC - 1),
                                    tile_position=(0, 32 * t4),
                                    skip_group_check=True,
                                )
                                nc.tensor.matmul(
                                    rs[32 * t4:32 * (t4 + 1), 0:N],
                                    lhsT=ones_sb[:],
                                    rhs=ptj[:, t4, :],
                                    start=(jc == 0), stop=(jc == NJC - 1),
                                    tile_position=(0, 32 * t4),
                                    skip_group_check=True,
                                )
                else:
                    for jc in range(NJC):
                        smp = psim.tile([128, 2048], F32, tag="sim")
                        for t4 in range(4):
                            nc.tensor.matmul(
                                smp[0:JC, 512 * t4:512 * t4 + N],
                                lhsT=qk_sb[2 + hg][32 * t4:32 * (t4 + 1), JC * jc:JC * (jc + 1)],
                                rhs=qk_sb[hg][32 * t4:32 * (t4 + 1), :],
                                start=True, stop=True,
                                tile_position=(32 * t4, 0),
                            )
                        if ab_noexp:
                            nc.scalar.copy(stub[:], smp[0:1, 0:8])
                            pts.append(pt_const)
                            continue
                        et = wpool.tile([JC, 4, N], FP16, tag=f"et{jc}")
                        sim_ap = smp[0:JC, :].rearrange("p (t x) -> p t x", t=4)[:, :, 0:N]
                        nc.scalar.activation(
                            et[:], sim_ap, EXP,
                            bias=msk_sb[:, NJC * w + jc:NJC * w + jc + 1],
                        )
                        pt = wpool.tile([JC, 4, N], FP16, tag=f"pt{jc}")
                        eb_ap = eb_sb[:, 4 * hg:4 * (hg + 1), jc, :]
                        ebmul(jc, pt[:], et[:], eb_ap)
                        pts.append(pt)

                if augv:
                    # two heads per PSUM tile at col offsets 0/64, M=33: the
                    # ones column of v lands the rowsum in psum row 32/96.
                    for hh in range(2):           # head pair (q-tile) in hg
                        q = 2 * hg + hh
                        PQ = pvrs.tile([128, 512], F32, tag="pvrs", name="PQ")
                        for h2 in range(2):       # head within pair
                            t4 = 2 * hh + h2
                            h = 4 * hg + t4
                            for jc in range(NJC):
                                nc.tensor.matmul(
                                    PQ[64 * h2:64 * h2 + DH + 1, 0:N],
                                    lhsT=v_sb[jc][:, h, :],
                                    rhs=pts[jc][:, t4, :],
                                    start=(jc == 0), stop=(jc == NJC - 1),
                                    tile_position=(0, 64 * h2),
                                    skip_group_check=True,
                                )
                        # 1/rowsum: recip whole column block (base-0 psum
                        # read; only rows 32 & 96 are consumed downstream)
                        rcp = wpool.tile([97, N], F32, tag="rcp", name="rcp")
                        nc.vector.reciprocal_approx_fast(rcp[:], PQ[0:97, 0:N])
                        rr = wpool.tile([96, N], F32, tag="rrq", name="rrq")
                        nc.gpsimd.partition_broadcast(rr[0:64, :], rcp[32:33, :])
                        nc.gpsimd.partition_broadcast(rr[64:96, :], rcp[96:97, :])
                        on = wpool.tile([96, N], FP16, tag=f"on{q}", name=f"on{q}")
                        nc.vector.tensor_mul(on[:], PQ[0:96, 0:N], rr[:])
                        on_sb.append(on)
                    return

                ptsum = None
                if rs_presum and not (ab_nopv or ab_nosim or ab_noexp):
                    # rowsum needs sum over all 294 j; sum the three j-chunks
                    # on DVE first so each head needs one ones-matmul, not 3
                    ptsum = wpool.tile([JC, 4, N], FP16, tag="ptsum")
                    nc.vector.tensor_add(ptsum[:], pts[0][:], pts[1][:])
                    nc.vector.tensor_add(ptsum[:], ptsum[:], pts[2][:])
                if pv_inline and sim_pairs and not (rs_presum or ab_nopv or ab_nosim or ab_noexp):
                    rr = wpool.tile([128, N], F32, tag="rr", name="rr")
                    nc.vector.reciprocal_approx_fast(rr[:], rs[:, 0:N])
                    on = wpool.tile([128, N], FP16, tag=f"on{hg}", name=f"onx{hg}")
                    nc.vector.tensor_mul(on[:], pv[:, 0:N], rr[:])
                    on_sb.append(on)
                    return
                pv_iters = [(0, [0])] if ab_nopv else [(t, list(range(NJC))) for t in range(4)]
                for t4, jcs in pv_iters:
                    h = 4 * hg + t4
                    for jc in jcs:
                        nc.tensor.matmul(
                            pv[32 * t4:32 * (t4 + 1), 0:N],
                            lhsT=v_sb[jc][:, 32 * h:32 * (h + 1)],
                            rhs=pts[jc][:, t4, :],
                            start=(jc == 0), stop=(jc == jcs[-1]),
                            tile_position=(0, 32 * t4),
                            skip_group_check=True,
                        )
                    if ptsum is not None:
                        nc.tensor.matmul(
                            rs[32 * t4:32 * (t4 + 1), 0:N],
                            lhsT=ones_sb[:],
                            rhs=ptsum[:, t4, :],
                            start=True, stop=True,
                            tile_position=(0, 32 * t4),
                            skip_group_check=True,
                        )
                        continue
                    for jc in jcs:
                        nc.tensor.matmul(
                            rs[32 * t4:32 * (t4 + 1), 0:N],
                            lhsT=ones_sb[:],
                            rhs=pts[jc][:, t4, :],
                            start=(jc == 0), stop=(jc == jcs[-1]),
                            tile_position=(0, 32 * t4),
                            skip_group_check=True,
                        )
                rr = wpool.tile([128, N], F32, tag="rr")
                nc.vector.reciprocal_approx_fast(rr[:], rs[:, 0:N])
                on = wpool.tile([128, N], FP16, tag=f"on{hg}")
                nc.vector.tensor_mul(on[:], pv[:, 0:N], rr[:])
                on_sb.append(on)

            return (lambda: do_hg(0), lambda: do_hg(1),
                    lambda: emit_out(w, on_sb))

        def emit_attn(w, qk_sb, v_sb):
            p0, p1, p2 = emit_attn_parts(w, qk_sb, v_sb)
            p0(); p1(); p2()

        def emit_out(w, on_sb):
            fo3 = wpool.tile([JC, NJC, DIM], ODT, tag="fo3", name="fo3") if one_dma else None
            nkc = 4 if augv else 2
            po01 = None

            def fo_copy(dst, src):
                if fo_eng == 'dve':
                    nc.vector.tensor_copy(dst, src)
                else:
                    nc.scalar.copy(dst, src)

            for ic in range(NJC):
                if po_pair and ic < 2:
                    # two 256-wide out-proj chunks share one PSUM bank
                    if ic == 0:
                        po01 = ps1.tile([128, 512], F32, tag="b1", name="po01")
                    po = po01[0:JC, 256 * ic:256 * ic + DIM]
                else:
                    po = ps1.tile([128, 512], F32, tag="b1", name="po")[0:JC, 0:DIM]
                for kc in range(nkc):
                    nc.tensor.matmul(
                        po,
                        lhsT=on_sb[kc][:, JC * ic:JC * (ic + 1)],
                        rhs=wout_sb[kc][:],
                        start=(kc == 0), stop=(kc == nkc - 1),
                        skip_group_check=po_pair,
                    )
                # copy each chunk as soon as its MMs are emitted (po_pair:
                # ic0+ic1 drain together after ic1)
                if po_pair and ic == 0:
                    continue
                if one_dma:
                    if po_pair and ic == 1:
                        fo_copy(fo3[:, 0:2, :], po01[0:JC, 0:2 * DIM])
                    else:
                        fo_copy(fo3[:, ic, :], po)
                else:
                    if po_pair and ic == 1:
                        fop = wpool.tile([JC, 2, DIM], ODT, tag="fop", name="fop")
                        fo_copy(fop[:], po01[0:JC, 0:2 * DIM])
                        nc.sync.dma_start(
                            out_d[w, 0:2 * JC, :].rearrange("(a p) e -> p a e", p=JC),
                            fop[:])
                        if timing and w == 0:
                            nc.sync.dma_start(
                                outx_d[0, 0:2 * JC, :].rearrange("(a p) e -> p a e", p=JC),
                                fop[:])
                        continue
                    fo = wpool.tile([JC, DIM], ODT, tag=f"fo{ic}", name=f"fo{ic}")
                    fo_copy(fo[:], po)
                    nc.sync.dma_start(out_d[w, JC * ic:JC * (ic + 1), :], fo[:])
                    if timing and w == 0:
                        nc.sync.dma_start(outx_d[0, JC * ic:JC * (ic + 1), :], fo[:])
            if one_dma:
                nc.sync.dma_start(
                    out_d[w].rearrange("(i p) e -> p i e", i=NJC), fo3[:])
                if timing and w == 0:
                    nc.sync.dma_start(
                        outx_d[0].rearrange("(i p) e -> p i e", i=NJC), fo3[:])
            if one_dma:
                nc.sync.dma_start(
                    out_d[w].rearrange("(i p) e -> p i e", i=NJC), fo3[:])
                if timing and w == 0:
                    nc.sync.dma_start(
                        outx_d[0].rearrange("(i p) e -> p i e", i=NJC), fo3[:])

        rep_ctx = tc.For_i(0, reps, 1) if reps > 1 else None
        if rep_ctx is not None:
            ctx.enter_context(rep_ctx)
        if pipe in ('split3', 'split4'):
            # split2 with the lag-2 outproj hoisted earlier in the window
            partsq = {}
            prev = None
            for w in range(n_wins):
                if prev is not None:
                    partsq[w - 1] = emit_attn_parts(w - 1, *prev)
                if pipe == 'split3' and w - 2 in partsq:
                    partsq[w - 2][2]()
                    del partsq[w - 2]
                qk = emit_qk(w)
                if w - 1 in partsq:
                    partsq[w - 1][0]()
                if pipe == 'split4' and w - 2 in partsq:
                    partsq[w - 2][2]()
                    del partsq[w - 2]
                v = emit_v(qk[0])
                if w - 1 in partsq:
                    partsq[w - 1][1]()
                prev = (qk[1], v)
            partsq[n_wins - 1] = emit_attn_parts(n_wins - 1, *prev)
            for w2 in sorted(partsq):
                if w2 < n_wins - 1:
                    partsq[w2][2]()
            partsq[n_wins - 1][0]()
            partsq[n_wins - 1][1]()
            partsq[n_wins - 1][2]()
        elif pipe == 'split2':
            # lag-1 attn interleave + lag-2 outproj: outproj of w-2 drains
            # while w-1's attention still owns the psum rings
            partsq = {}
            prev = None
            for w in range(n_wins):
                if xt_pre and xt_pair and w % 2 == 0:
                    lag = 2 * max(1, int(xt_pre) // 2)
                    if w + lag < n_wins:
                        load_xt(w + lag, prefetch=True)
                    if w == 0:
                        for b2 in range(2, min(lag, n_wins), 2):
                            load_xt(b2, prefetch=True)
                if prev is not None:
                    partsq[w - 1] = emit_attn_parts(w - 1, *prev)
                qk = emit_qk(w)
                if w - 1 in partsq:
                    partsq[w - 1][0]()
                v = emit_v(qk[0])
                if w - 1 in partsq:
                    partsq[w - 1][1]()
                if w - 2 in partsq:
                    partsq[w - 2][2]()
                    del partsq[w - 2]
                prev = (qk[1], v)
            partsq[n_wins - 1] = emit_attn_parts(n_wins - 1, *prev)
            if n_wins - 2 in partsq:
                partsq[n_wins - 2][2]()
            partsq[n_wins - 1][0]()
            partsq[n_wins - 1][1]()
            partsq[n_wins - 1][2]()
        elif pipe == 'split':
            # 1-stage pipeline with split QKV emission: qk of w+1 lands
            # before attn(w) hg0, v of w+1 between hg0 and hg1, so PE
            # fill-work is spread through the attention chain stalls
            prev = None
            for w in range(n_wins):
                parts = emit_attn_parts(w - 1, *prev) if prev is not None else None
                qk = emit_qk(w)
                if parts is not None:
                    parts[0]()          # attn hg0 of w-1
                v = emit_v(qk[0])
                if parts is not None:
                    parts[1]()          # attn hg1 of w-1
                    parts[2]()          # outproj of w-1
                prev = (qk[1], v)
            parts = emit_attn_parts(n_wins - 1, *prev)
            parts[0](); parts[1](); parts[2]()
        elif pipe:
            # 1-stage software pipeline: QKV of window w+1 is emitted (and
            # thus prioritized) ahead of attention of window w, so the PE
            # fills softmax-chain stalls with the next window's projections
            prev = None
            for w in range(n_wins):
                cur = emit_qkv(w)
                if prev is not None:
                    emit_attn(w - 1, *prev)
                prev = cur
            emit_attn(n_wins - 1, *prev)
        else:
            for w in range(n_wins):
                qk_sb, v_sb = emit_qkv(w)
                emit_attn(w, qk_sb, v_sb)

    nc.compile()
    return nc


def host_prep(x, mask, w_qkv, w_out, bias_table, augv=None):
    """Build per-core input maps (numpy only)."""
    x = np.asarray(x, dtype=np.float32)
    mask = np.asarray(mask)
    w_qkv = np.asarray(w_qkv, dtype=np.float32)
    w_out = np.asarray(w_out, dtype=np.float32)
    bias_table = np.asarray(bias_table, dtype=np.float32)

    # x: (B, L, X, Y, W1, W2, C) -> windows (B,X,Y) x [C, N]
    xr = np.ascontiguousarray(x.transpose(0, 2, 3, 1, 4, 5, 6)).reshape(NWIN, N, DIM)
    xt = np.ascontiguousarray(xr.transpose(0, 2, 1)).reshape(NWIN, 2, 128, N).astype(np.float16)

    # mask: (B, X, Y, W1, W2, 1, L) -> (B,X,Y) x N with token order (l, w1, w2)
    m = np.ascontiguousarray(mask.transpose(0, 1, 2, 5, 6, 3, 4)).reshape(NWIN, N)
    maskadd = np.where(m == 0, np.float32(MASK_NEG), np.float32(0.0)).astype(np.float32)

    # exp(bias) transposed: ebT[h, j, i] = exp(bias[i, j, h])
    ri = _rel_pos_index(AGENT, WIN, WIN)
    bias = bias_table[ri]                       # (N, N, H) [i, j, h]
    ebT = np.exp(bias.transpose(2, 1, 0))       # (H, j, i)
    eb_host = np.ascontiguousarray(
        ebT.reshape(HEADS, NJC, JC, N).transpose(2, 0, 1, 3)
    ).astype(np.float16)                # (JC, H, NJC, N)

    wq = w_qkv.copy()
    wq[0:DIM] *= np.float32(SCALE)
    wqkvT = np.ascontiguousarray(wq.T).reshape(2, 128, 3 * DIM).astype(np.float16)
    if augv is None:
        augv = bool(BEST_CFG.get("augv"))
    if augv:
        # [4 head-pair tiles, 97 rows, DIM]: rows 0-31 head 2q, rows 64-95
        # head 2q+1, rows 32-63 & 96 zero (match the PV psum row layout)
        wt = np.asarray(w_out.T, dtype=np.float32)     # [c, e]
        woutT = np.zeros((4, 96, DIM), dtype=np.float16)
        for q in range(4):
            woutT[q, 0:32] = wt[64 * q:64 * q + 32].astype(np.float16)
            woutT[q, 64:96] = wt[64 * q + 32:64 * q + 64].astype(np.float16)
    else:
        woutT = np.ascontiguousarray(w_out.T).reshape(2, 128, DIM).astype(np.float16)

    in_maps = []
    for core in range(NCORES):
        ws = slice(WPC * core, WPC * (core + 1))
        mm = maskadd[ws].reshape(WPC, NJC, JC).transpose(2, 0, 1).reshape(JC, WPC * NJC)
        in_maps.append({
            "xt": np.ascontiguousarray(xt[ws]),
            "msk": np.ascontiguousarray(mm),
            "eb": eb_host,
            "wqkv": wqkvT,
            "wout": woutT,
        })
    return in_maps


def assemble_output(core_outs):
    """core_outs: list of [WPC, N, DIM] arrays -> full (B, L, X, Y, W1, W2, C)."""
    out = np.concatenate([np.asarray(o) for o in core_outs], axis=0).astype(np.float32)
    out = out.reshape(B, X, Y, AGENT, WIN, WIN, DIM)
    return np.ascontiguousarray(out.transpose(0, 3, 1, 2, 4, 5, 6)).astype(np.float32)


_NC_CACHE = {}


def _get_nc(n_wins=WPC):
    if n_wins not in _NC_CACHE:
        _NC_CACHE[n_wins] = build_graph(n_wins, **BEST_CFG)
    return _NC_CACHE[n_wins]


BEST_CFG = dict(wbufs=4, v_eng="dve", out16=True, sim_pairs=True, fo_eng="act", pipe='split2', one_dma=True, xt_pair=True, qk_eng='act', xt_pre=True)


def kernel(x, mask, w_qkv, w_out, bias_table):
    in_maps = host_prep(x, mask, w_qkv, w_out, bias_table)
    nc = _get_nc(WPC)
    res = run_bass_kernel_spmd(nc, in_maps, core_ids=list(range(NCORES)))
    core_outs = [res.results[i]["out"] for i in range(NCORES)]
    return assemble_output(core_outs)



# revision 55
# speedup vs baseline: 1.0245x; 1.0245x over previous
"""Trainium2 Bass kernel for windowed sparse attention (nn_Attention_17703855194428).

Reference computation (per window w of 128 = B*X*Y, tokens N=294 = L*W1*W2):
    qkv = x_w @ w_qkv.T ; q,k,v heads (8 heads x 32 dim), q scaled by 1/sqrt(32)
    sim = q @ k.T + rel_pos_bias ; masked cols -> -1e9 ; softmax over keys
    out = (attn @ v) @ w_out.T

Sharding: pure data parallel over the 128 independent windows -> 16 windows
per NeuronCore, weights/bias replicated. No collectives.

Kernel layout strategy (per core, per window; all matmul inputs fp16,
fp32 PSUM accumulate; host pre-computes exp(rel-pos-bias) transposed and the
additive mask in the kernel's SBUF layouts):
    xT [C=256, N=294] (channels on partitions) ->
    q,k as [E, N] (head-dim on partitions), v as [N, E] (tokens on partitions)
    simT[j, i] = sum_d k[d,j] q[d,i] via row-tiled (K=32) matmuls, 2 heads
        per 2-bank psum tile, double-buffered (sim_pairs)
    P_T = exp(simT + mask_j) * exp(bias)_T   (one ACT exp with per-partition
        mask bias reading across psum banks + one DVE fp16 multiply)
    outU.T[hd, i] = sum_j v[j, hd] P_T[j, i] via col-tiled (M=32) matmuls
    rowsum broadcast to all 32 head rows for free via col-tiled ones-matmul
    1/rowsum via DVE reciprocal_approx_fast; normalize is one DVE multiply
    out = (outU.T * (1/rowsum)).T @ w_out.T via K=hd matmuls -> [tok, C],
    written fp16 (host upcasts to fp32)

Pipelining (pipe='split2'): QKV emission of window w+1 is split around
the attention of window w (qk ahead of hg0, v between hg0 and hg1), and
the out-projection runs at lag 2 (outproj of w-2 after hg1 of w-1), so
every engine has off-chain work during sim->exp->mul->pv stalls.
I/O batching: one output DMA per window (fo gathered in a [98, 3, 256]
tile, dram viewed as (i p) e -> p i e) and one xt load per window-pair,
prefetched one pair ahead (xt_pre — the pair DMA otherwise stalls the
first q/k matmuls of every even window; -1.4% HW-measured).
Engine placement (HW-measured): all four q/k psum->sbuf copies on ACT
(qk_eng='act'), v copies + eb-mul + recip + normalize on DVE, fo on ACT.

Measured on HW (differential For_i timing): ~248-298 us per core for 16
windows depending on the session (vs ~327-344 us for the session-start
baseline; HW sessions drift +-16 % between processes — only within-
process interleaved A/B comparisons are trustworthy). Engines are nearly
balanced (PE ~12.5 us/win engine-busy, ACT ~11, DVE ~9 in the cost-model
sim); the toolchain serializes every Ldweights+Matmult pair
(--enable-ldw-opt=false), so matmul count matters as much as F-cycles.
Rejected on HW measurement: rowsum-in-PV via augmented V (normalize
broadcast costs exceed the PE win; gpsimd partition_broadcast also reads
physical partition 0, and PV leaves psum rows 33-63 uninitialized),
pool-engine eb-muls (+5% — GPSIMD has no PSUM port and shares the DVE
SBUF port), fp8 DoubleRow (needs cross-partition pair-interleaved
layouts), rs_presum / merged eb-muls / paired out-proj psum banks /
fo_dve / v_act (+3-9% each: chain latency beats instruction count).
"""

import numpy as np
from contextlib import ExitStack

import concourse.bass as bass
import concourse.bacc as bacc
import concourse.mybir as mybir
from concourse import tile
from concourse.bass_utils import run_bass_kernel_spmd

import ml_dtypes

F32 = mybir.dt.float32
F32R = mybir.dt.float32r
BF16 = mybir.dt.bfloat16
FP16 = mybir.dt.float16
EXP = mybir.ActivationFunctionType.Exp

# Problem constants (hardcoded per harness contract)
B, AGENT, X, Y, WIN, DIM, HEADS, DH = 2, 6, 8, 8, 7, 256, 8, 32
N = AGENT * WIN * WIN            # 294 tokens per window
NWIN = B * X * Y                 # 128 windows
NCORES = 8
WPC = NWIN // NCORES             # 16 windows per core
JC = 98                          # key-chunk size (294 = 3*98)
NJC = 3
SCALE = DH ** -0.5
MASK_NEG = -1e9


def _rel_pos_index(L, Wh, Ww):
    coords = np.stack(np.meshgrid(np.arange(L), np.arange(Wh), np.arange(Ww), indexing="ij"))
    cf = coords.reshape(3, -1)
    rel = cf[:, :, None] - cf[:, None, :]
    rel = rel.transpose(1, 2, 0).astype(np.int64)
    rel[..., 0] += L - 1
    rel[..., 1] += Wh - 1
    rel[..., 2] += Ww - 1
    rel[..., 0] *= (2 * Wh - 1) * (2 * Ww - 1)
    rel[..., 1] *= 2 * Ww - 1
    return rel.sum(-1)  # (N, N) [i, j]


def build_graph(n_wins=WPC, wbufs=2, pvrs_bufs=2, misc_bufs=2, trace_sim=False, reps=1,
                v_eng='act', fo_eng='dve', gp_jc=(), sim_pairs=False, timing=False, out16=False, ab_noexp=False, ab_nosim=False, ab_nopv=False, pipe=False, sim_bufs=2, rs_presum=False,
                mul_eng='dve', one_dma=False, xt_pair=False, qk_eng='split',
                augv=False, po_pair=False, mul4=False, xt_pre=False, pv_inline=False, qk_order=False):
    nc = bacc.Bacc(None)
    n_xt = 2 if timing else n_wins
    xt_d = nc.declare_dram_parameter("xt", [n_xt, 2, 128, N], FP16, isOutput=False)
    msk_d = nc.declare_dram_parameter("msk", [JC, n_wins * NJC], F32, isOutput=False)
    eb_d = nc.declare_dram_parameter("eb", [JC, HEADS, NJC, N], FP16, isOutput=False)
    wqkv_d = nc.declare_dram_parameter("wqkv", [2, 128, 3 * DIM], FP16, isOutput=False)
    if augv:
        wout_d = nc.declare_dram_parameter("wout", [4, 96, DIM], FP16, isOutput=False)
    else:
        wout_d = nc.declare_dram_parameter("wout", [2, 128, DIM], FP16, isOutput=False)
    ODT = FP16 if out16 else F32
    if timing:
        out_d = nc.dram_tensor("oscr", [n_wins, N, DIM], ODT)
        outx_d = nc.declare_dram_parameter("out", [1, N, DIM], ODT, isOutput=True)
    else:
        out_d = nc.declare_dram_parameter("out", [n_wins, N, DIM], ODT, isOutput=True)
        outx_d = None

    with tile.TileContext(nc, trace_sim=trace_sim) as tc, ExitStack() as ctx:
        cpool = ctx.enter_context(tc.tile_pool(name="consts", bufs=1))
        wpool = ctx.enter_context(tc.tile_pool(name="work", bufs=wbufs))
        # one xt slot per window: slot reuse on DMA-written tiles piles up
        # sync waits beyond what DMA descriptors support
        xpool = ctx.enter_context(tc.tile_pool(name="xin", bufs=n_wins))
        if sim_pairs:
            psim = ctx.enter_context(tc.tile_pool(name="psim", bufs=sim_bufs, space="PSUM"))
        else:
            psim = ctx.enter_context(tc.tile_pool(name="psim", bufs=1, space="PSUM"))
        pvrs = ctx.enter_context(tc.tile_pool(name="pvrs", bufs=pvrs_bufs, space="PSUM"))
        ps1 = ctx.enter_context(tc.tile_pool(name="ps1", bufs=misc_bufs, space="PSUM"))

        # ---- replicated constants ----
        wqkv_sb = []
        for c in range(2):
            t = cpool.tile([128, 3 * DIM], FP16, tag=f"wqkv{c}")
            nc.sync.dma_start(t[:], wqkv_d[c])
            wqkv_sb.append(t)
        wout_sb = []
        for c in range(4 if augv else 2):
            t = cpool.tile([96, DIM] if augv else [128, DIM], FP16,
                           tag=f"wout{c}", name=f"wout{c}")
            nc.sync.dma_start(t[:], wout_d[c])
            wout_sb.append(t)
        eb_sb = cpool.tile([JC, HEADS, NJC, N], FP16, tag="eb")
        nc.sync.dma_start(eb_sb[:], eb_d[:])
        msk_sb = cpool.tile([JC, n_wins * NJC], F32, tag="msk")
        nc.sync.dma_start(msk_sb[:], msk_d[:])
        ones_sb = cpool.tile([JC, 32], FP16, tag="ones")
        nc.vector.memset(ones_sb[:], 1.0)

        if ab_noexp or ab_nosim:
            pt_const = cpool.tile([JC, 4, N], FP16, tag="ptc")
            nc.vector.memset(pt_const[:], 0.01)
            stub = cpool.tile([1, 8], F32, tag="stub")

        # warm-up touches: absorb the one-time const-DMA waits into throwaway
        # instructions so steady-state ops stay within the per-instruction
        # sync-wait budget
        scr_a = cpool.tile([JC, 1], F32, tag="scr_a")
        nc.scalar.copy(scr_a[:], msk_sb[:, 0:1])
        scr_v = cpool.tile([JC, 1], FP16, tag="scr_v")
        nc.vector.tensor_copy(scr_v[:], eb_sb[:, 0, 0, 0:1])

        xt_cache = {}

        def load_xt(w, prefetch=False):
            if xt_pair:
                base = w - (w % 2)
                if base not in xt_cache:
                    # keep earlier pairs resident (prefetch runs ahead)
                    keep = max(2, (int(xt_pre) if xt_pre else 0) // 2 + 1)
                    while len(xt_cache) >= keep:
                        del xt_cache[min(xt_cache)]
                    ts = []
                    for c in range(2):
                        t = xpool.tile([128, 2, N], FP16, tag=f"xt{c}", name=f"xtp{c}")
                        hi = min(base + 2, n_wins)
                        src = xt_d[base % n_xt:(base % n_xt) + (hi - base), c]
                        nc.sync.dma_start(
                            t[:, 0:hi - base, :], src.rearrange("w p n -> p w n"))
                        ts.append(t)
                    xt_cache[base] = ts
                if prefetch:
                    return None
                xt_t = [xt_cache[base][c][:, w - base, :] for c in range(2)]
            else:
                if prefetch:
                    return None
                xt_t = []
                for c in range(2):
                    t = xpool.tile([128, N], FP16, tag=f"xt{c}")
                    nc.sync.dma_start(t[:], xt_d[w % n_xt, c])
                    xt_t.append(t)
            return xt_t

        def emit_qk(w):
            xt_t = load_xt(w)
            qk_sb = [None] * 4
            # k-low, q-low first: hg0's sim can start after two copies, and
            # the k Ldweights is ready before the q rhs
            porder = (2, 0, 3, 1) if qk_order else (0, 1, 2, 3)
            for p in porder:
                ps = ps1.tile([128, 512], F32, tag="b1")
                for c in range(2):
                    nc.tensor.matmul(
                        ps[:, 0:N],
                        lhsT=wqkv_sb[c][:, 128 * p:128 * (p + 1)],
                        rhs=xt_t[c][:],
                        start=(c == 0), stop=(c == 1),
                    )
                t = wpool.tile([128, N], FP16, tag=f"qk{p}")
                use_act = (p < 2) if qk_eng == 'split' else (qk_eng == 'act')
                if use_act:
                    nc.scalar.copy(t[:], ps[:, 0:N])
                else:
                    nc.vector.tensor_copy(t[:], ps[:, 0:N])
                qk_sb[p] = t
            return xt_t, qk_sb

        def emit_v(xt_t):
            v_sb = []
            for j in range(NJC):
                ps = ps1.tile([128, 512], F32, tag="b1")
                for c in range(2):
                    nc.tensor.matmul(
                        ps[0:JC, 0:DIM],
                        lhsT=xt_t[c][:, JC * j:JC * (j + 1)],
                        rhs=wqkv_sb[c][:, 2 * DIM:3 * DIM],
                        start=(c == 0), stop=(c == 1),
                    )
                if augv:
                    # per-head 33rd column of ones: rowsum rides the PV matmul
                    t = wpool.tile([JC, HEADS, DH + 1], FP16, tag=f"v{j}",
                                   name=f"v{j}")
                    dst = t[:, :, 0:DH]
                    if v_eng == 'act':
                        nc.scalar.copy(dst, ps[0:JC, 0:DIM])
                    else:
                        nc.vector.tensor_copy(dst, ps[0:JC, 0:DIM])
                    nc.vector.memset(t[:, :, DH:DH + 1], 1.0)
                else:
                    t = wpool.tile([JC, DIM], FP16, tag=f"v{j}", name=f"v{j}")
                    if v_eng == 'act':
                        nc.scalar.copy(t[:], ps[0:JC, 0:DIM])
                    else:
                        nc.vector.tensor_copy(t[:], ps[0:JC, 0:DIM])
                v_sb.append(t)
            return v_sb

        def emit_qkv(w):
            xt_t, qk_sb = emit_qk(w)
            return qk_sb, emit_v(xt_t)

        def emit_attn_parts(w, qk_sb, v_sb):
            def ebmul(k, out_ap, a_ap, b_ap):
                eng = mul_eng if mul_eng != 'mix' else ('pool' if k % 2 else 'dve')
                if eng == 'pool':
                    nc.gpsimd.tensor_mul(out_ap, a_ap, b_ap)
                else:
                    nc.vector.tensor_mul(out_ap, a_ap, b_ap)
            on_sb = []

            def do_hg(hg):
                if not augv:
                    pv = pvrs.tile([128, 512], F32, tag="pvrs", name="pv")
                    rs = pvrs.tile([128, 512], F32, tag="pvrs", name="rs")
                pts = []
                if ab_nosim:
                    pts = [pt_const] * NJC
                elif sim_pairs:
                    for jc in range(NJC):
                        ptj = wpool.tile([JC, 4, N], FP16, tag=f"pt{jc}")
                        et4 = wpool.tile([JC, 4, N], FP16, tag=f"et4{jc}",
                                         name=f"et4{jc}") if mul4 else None
                        for sg in range(2):
                            smp = psim.tile([128, 1024], F32, tag="sim")
                            for i2 in range(2):
                                t4 = 2 * sg + i2
                                nc.tensor.matmul(
                                    smp[0:JC, 512 * i2:512 * i2 + N],
                                    lhsT=qk_sb[2 + hg][32 * t4:32 * (t4 + 1), JC * jc:JC * (jc + 1)],
                                    rhs=qk_sb[hg][32 * t4:32 * (t4 + 1), :],
                                    start=True, stop=True,
                                    tile_position=(32 * t4, 0),
                                )
                            et = (et4[:, 2 * sg:2 * sg + 2, :] if mul4 else
                                  wpool.tile([JC, 2, N], FP16, tag=f"et{jc}{sg}",
                                             name=f"et{jc}{sg}")[:])
                            sim_ap = smp[0:JC, :].rearrange("p (t x) -> p t x", t=2)[:, :, 0:N]
                            nc.scalar.activation(
                                et, sim_ap, EXP,
                                bias=msk_sb[:, NJC * w + jc:NJC * w + jc + 1],
                            )
                            if not mul4:
                                eb_ap = eb_sb[:, 4 * hg + 2 * sg:4 * hg + 2 * sg + 2, jc, :]
                                ebmul(2 * jc + sg, ptj[:, 2 * sg:2 * sg + 2, :], et, eb_ap)
                        if mul4:
                            eb_ap = eb_sb[:, 4 * hg:4 * (hg + 1), jc, :]
                            ebmul(jc, ptj[:], et4[:], eb_ap)
                        pts.append(ptj)
                        if pv_inline and not (rs_presum or ab_nopv):
                            for t4 in range(4):
                                h = 4 * hg + t4
                                nc.tensor.matmul(
                                    pv[32 * t4:32 * (t4 + 1), 0:N],
                                    lhsT=v_sb[jc][:, 32 * h:32 * (h + 1)],
                                    rhs=ptj[:, t4, :],
                                    start=(jc == 0), stop=(jc == NJC - 1),
                                    tile_position=(0, 32 * t4),
                                    skip_group_check=True,
                                )
                                nc.tensor.matmul(
                                    rs[32 * t4:32 * (t4 + 1), 0:N],
                                    lhsT=ones_sb[:],
                                    rhs=ptj[:, t4, :],
                                    start=(jc == 0), stop=(jc == NJC - 1),
                                    tile_position=(0, 32 * t4),
                                    skip_group_check=True,
                                )
                else:
                    for jc in range(NJC):
                        smp = psim.tile([128, 2048], F32, tag="sim")
                        for t4 in range(4):
                            nc.tensor.matmul(
                                smp[0:JC, 512 * t4:512 * t4 + N],
                                lhsT=qk_sb[2 + hg][32 * t4:32 * (t4 + 1), JC * jc:JC * (jc + 1)],
                                rhs=qk_sb[hg][32 * t4:32 * (t4 + 1), :],
                                start=True, stop=True,
                                tile_position=(32 * t4, 0),
                            )
                        if ab_noexp:
                            nc.scalar.copy(stub[:], smp[0:1, 0:8])
                            pts.append(pt_const)
                            continue
                        et = wpool.tile([JC, 4, N], FP16, tag=f"et{jc}")
                        sim_ap = smp[0:JC, :].rearrange("p (t x) -> p t x", t=4)[:, :, 0:N]
                        nc.scalar.activation(
                            et[:], sim_ap, EXP,
                            bias=msk_sb[:, NJC * w + jc:NJC * w + jc + 1],
                        )
                        pt = wpool.tile([JC, 4, N], FP16, tag=f"pt{jc}")
                        eb_ap = eb_sb[:, 4 * hg:4 * (hg + 1), jc, :]
                        ebmul(jc, pt[:], et[:], eb_ap)
                        pts.append(pt)

                if augv:
                    # two heads per PSUM tile at col offsets 0/64, M=33: the
                    # ones column of v lands the rowsum in psum row 32/96.
                    for hh in range(2):           # head pair (q-tile) in hg
                        q = 2 * hg + hh
                        PQ = pvrs.tile([128, 512], F32, tag="pvrs", name="PQ")
                        for h2 in range(2):       # head within pair
                            t4 = 2 * hh + h2
                            h = 4 * hg + t4
                            for jc in range(NJC):
                                nc.tensor.matmul(
                                    PQ[64 * h2:64 * h2 + DH + 1, 0:N],
                                    lhsT=v_sb[jc][:, h, :],
                                    rhs=pts[jc][:, t4, :],
                                    start=(jc == 0), stop=(jc == NJC - 1),
                                    tile_position=(0, 64 * h2),
                                    skip_group_check=True,
                                )
                        # 1/rowsum: recip whole column block (base-0 psum
                        # read; only rows 32 & 96 are consumed downstream)
                        rcp = wpool.tile([97, N], F32, tag="rcp", name="rcp")
                        nc.vector.reciprocal_approx_fast(rcp[:], PQ[0:97, 0:N])
                        rr = wpool.tile([96, N], F32, tag="rrq", name="rrq")
                        nc.gpsimd.partition_broadcast(rr[0:64, :], rcp[32:33, :])
                        nc.gpsimd.partition_broadcast(rr[64:96, :], rcp[96:97, :])
                        on = wpool.tile([96, N], FP16, tag=f"on{q}", name=f"on{q}")
                        nc.vector.tensor_mul(on[:], PQ[0:96, 0:N], rr[:])
                        on_sb.append(on)
                    return

                ptsum = None
                if rs_presum and not (ab_nopv or ab_nosim or ab_noexp):
                    # rowsum needs sum over all 294 j; sum the three j-chunks
                    # on DVE first so each head needs one ones-matmul, not 3
                    ptsum = wpool.tile([JC, 4, N], FP16, tag="ptsum")
                    nc.vector.tensor_add(ptsum[:], pts[0][:], pts[1][:])
                    nc.vector.tensor_add(ptsum[:], ptsum[:], pts[2][:])
                if pv_inline and sim_pairs and not (rs_presum or ab_nopv or ab_nosim or ab_noexp):
                    rr = wpool.tile([128, N], F32, tag="rr", name="rr")
                    nc.vector.reciprocal_approx_fast(rr[:], rs[:, 0:N])
                    on = wpool.tile([128, N], FP16, tag=f"on{hg}", name=f"onx{hg}")
                    nc.vector.tensor_mul(on[:], pv[:, 0:N], rr[:])
                    on_sb.append(on)
                    return
                pv_iters = [(0, [0])] if ab_nopv else [(t, list(range(NJC))) for t in range(4)]
                for t4, jcs in pv_iters:
                    h = 4 * hg + t4
                    for jc in jcs:
                        nc.tensor.matmul(
                            pv[32 * t4:32 * (t4 + 1), 0:N],
                            lhsT=v_sb[jc][:, 32 * h:32 * (h + 1)],
                            rhs=pts[jc][:, t4, :],
                            start=(jc == 0), stop=(jc == jcs[-1]),
                            tile_position=(0, 32 * t4),
                            skip_group_check=True,
                        )
                    if ptsum is not None:
                        nc.tensor.matmul(
                            rs[32 * t4:32 * (t4 + 1), 0:N],
                            lhsT=ones_sb[:],
                            rhs=ptsum[:, t4, :],
                            start=True, stop=True,
                            tile_position=(0, 32 * t4),
                            skip_group_check=True,
                        )
                        continue
                    for jc in jcs:
                        nc.tensor.matmul(
                            rs[32 * t4:32 * (t4 + 1), 0:N],
                            lhsT=ones_sb[:],
                            rhs=pts[jc][:, t4, :],
                            start=(jc == 0), stop=(jc == jcs[-1]),
                            tile_position=(0, 32 * t4),
                            skip_group_check=True,
                        )
                rr = wpool.tile([128, N], F32, tag="rr")
                nc.vector.reciprocal_approx_fast(rr[:], rs[:, 0:N])
                on = wpool.tile([128, N], FP16, tag=f"on{hg}")
                nc.vector.tensor_mul(on[:], pv[:, 0:N], rr[:])
                on_sb.append(on)

            return (lambda: do_hg(0), lambda: do_hg(1),
                    lambda: emit_out(w, on_sb))

        def emit_attn(w, qk_sb, v_sb):
            p0, p1, p2 = emit_attn_parts(w, qk_sb, v_sb)
            p0(); p1(); p2()

        def emit_out(w, on_sb):
            fo3 = wpool.tile([JC, NJC, DIM], ODT, tag="fo3", name="fo3") if one_dma else None
            nkc = 4 if augv else 2
            po01 = None

            def fo_copy(dst, src):
                if fo_eng == 'dve':
                    nc.vector.tensor_copy(dst, src)
                else:
                    nc.scalar.copy(dst, src)

            for ic in range(NJC):
                if po_pair and ic < 2:
                    # two 256-wide out-proj chunks share one PSUM bank
                    if ic == 0:
                        po01 = ps1.tile([128, 512], F32, tag="b1", name="po01")
                    po = po01[0:JC, 256 * ic:256 * ic + DIM]
                else:
                    po = ps1.tile([128, 512], F32, tag="b1", name="po")[0:JC, 0:DIM]
                for kc in range(nkc):
                    nc.tensor.matmul(
                        po,
                        lhsT=on_sb[kc][:, JC * ic:JC * (ic + 1)],
                        rhs=wout_sb[kc][:],
                        start=(kc == 0), stop=(kc == nkc - 1),
                        skip_group_check=po_pair,
                    )
                # copy each chunk as soon as its MMs are emitted (po_pair:
                # ic0+ic1 drain together after ic1)
                if po_pair and ic == 0:
                    continue
                if one_dma:
                    if po_pair and ic == 1:
                        fo_copy(fo3[:, 0:2, :], po01[0:JC, 0:2 * DIM])
                    else:
                        fo_copy(fo3[:, ic, :], po)
                else:
                    if po_pair and ic == 1:
                        fop = wpool.tile([JC, 2, DIM], ODT, tag="fop", name="fop")
                        fo_copy(fop[:], po01[0:JC, 0:2 * DIM])
                        nc.sync.dma_start(
                            out_d[w, 0:2 * JC, :].rearrange("(a p) e -> p a e", p=JC),
                            fop[:])
                        if timing and w == 0:
                            nc.sync.dma_start(
                                outx_d[0, 0:2 * JC, :].rearrange("(a p) e -> p a e", p=JC),
                                fop[:])
                        continue
                    fo = wpool.tile([JC, DIM], ODT, tag=f"fo{ic}", name=f"fo{ic}")
                    fo_copy(fo[:], po)
                    nc.sync.dma_start(out_d[w, JC * ic:JC * (ic + 1), :], fo[:])
                    if timing and w == 0:
                        nc.sync.dma_start(outx_d[0, JC * ic:JC * (ic + 1), :], fo[:])
            if one_dma:
                nc.sync.dma_start(
                    out_d[w].rearrange("(i p) e -> p i e", i=NJC), fo3[:])
                if timing and w == 0:
                    nc.sync.dma_start(
                        outx_d[0].rearrange("(i p) e -> p i e", i=NJC), fo3[:])
            if one_dma:
                nc.sync.dma_start(
                    out_d[w].rearrange("(i p) e -> p i e", i=NJC), fo3[:])
                if timing and w == 0:
                    nc.sync.dma_start(
                        outx_d[0].rearrange("(i p) e -> p i e", i=NJC), fo3[:])

        rep_ctx = tc.For_i(0, reps, 1) if reps > 1 else None
        if rep_ctx is not None:
            ctx.enter_context(rep_ctx)
        if pipe == 'split5':
            # full QKV of w ahead of attn of w-1 (like pipe=True), but with
            # split2's lag-2 outproj
            partsq = {}
            prev = None
            for w in range(n_wins):
                if xt_pre and xt_pair and w % 2 == 0:
                    lag = 2 * max(1, int(xt_pre) // 2)
                    if w + lag < n_wins:
                        load_xt(w + lag, prefetch=True)
                if prev is not None:
                    partsq[w - 1] = emit_attn_parts(w - 1, *prev)
                qk = emit_qk(w)
                v = emit_v(qk[0])
                if w - 1 in partsq:
                    partsq[w - 1][0]()
                    partsq[w - 1][1]()
                if w - 2 in partsq:
                    partsq[w - 2][2]()
                    del partsq[w - 2]
                prev = (qk[1], v)
            partsq[n_wins - 1] = emit_attn_parts(n_wins - 1, *prev)
            if n_wins - 2 in partsq:
                partsq[n_wins - 2][2]()
            partsq[n_wins - 1][0]()
            partsq[n_wins - 1][1]()
            partsq[n_wins - 1][2]()
        elif pipe in ('split3', 'split4'):
            # split2 with the lag-2 outproj hoisted earlier in the window
            partsq = {}
            prev = None
            for w in range(n_wins):
                if prev is not None:
                    partsq[w - 1] = emit_attn_parts(w - 1, *prev)
                if pipe == 'split3' and w - 2 in partsq:
                    partsq[w - 2][2]()
                    del partsq[w - 2]
                qk = emit_qk(w)
                if w - 1 in partsq:
                    partsq[w - 1][0]()
                if pipe == 'split4' and w - 2 in partsq:
                    partsq[w - 2][2]()
                    del partsq[w - 2]
                v = emit_v(qk[0])
                if w - 1 in partsq:
                    partsq[w - 1][1]()
                prev = (qk[1], v)
            partsq[n_wins - 1] = emit_attn_parts(n_wins - 1, *prev)
            for w2 in sorted(partsq):
                if w2 < n_wins - 1:
                    partsq[w2][2]()
            partsq[n_wins - 1][0]()
            partsq[n_wins - 1][1]()
            partsq[n_wins - 1][2]()
        elif pipe == 'split2':
            # lag-1 attn interleave + lag-2 outproj: outproj of w-2 drains
            # while w-1's attention still owns the psum rings
            partsq = {}
            prev = None
            for w in range(n_wins):
                if xt_pre and xt_pair and w % 2 == 0:
                    lag = 2 * max(1, int(xt_pre) // 2)
                    if w + lag < n_wins:
                        load_xt(w + lag, prefetch=True)
                    if w == 0:
                        for b2 in range(2, min(lag, n_wins), 2):
                            load_xt(b2, prefetch=True)
                if prev is not None:
                    partsq[w - 1] = emit_attn_parts(w - 1, *prev)
                qk = emit_qk(w)
                if w - 1 in partsq:
                    partsq[w - 1][0]()
                v = emit_v(qk[0])
                if w - 1 in partsq:
                    partsq[w - 1][1]()
                if w - 2 in partsq:
                    partsq[w - 2][2]()
                    del partsq[w - 2]
                prev = (qk[1], v)
            partsq[n_wins - 1] = emit_attn_parts(n_wins - 1, *prev)
            if n_wins - 2 in partsq:
                partsq[n_wins - 2][2]()
            partsq[n_wins - 1][0]()
            partsq[n_wins - 1][1]()
            partsq[n_wins - 1][2]()
        elif pipe == 'split':
            # 1-stage pipeline with split QKV emission: qk of w+1 lands
            # before attn(w) hg0, v of w+1 between hg0 and hg1, so PE
            # fill-work is spread through the attention chain stalls
            prev = None
            for w in range(n_wins):
                parts = emit_attn_parts(w - 1, *prev) if prev is not None else None
                qk = emit_qk(w)
                if parts is not None:
                    parts[0]()          # attn hg0 of w-1
                v = emit_v(qk[0])
                if parts is not None:
                    parts[1]()          # attn hg1 of w-1
                    parts[2]()          # outproj of w-1
                prev = (qk[1], v)
            parts = emit_attn_parts(n_wins - 1, *prev)
            parts[0](); parts[1](); parts[2]()
        elif pipe:
            # 1-stage software pipeline: QKV of window w+1 is emitted (and
            # thus prioritized) ahead of attention of window w, so the PE
            # fills softmax-chain stalls with the next window's projections
            prev = None
            for w in range(n_wins):
                cur = emit_qkv(w)
                if prev is not None:
                    emit_attn(w - 1, *prev)
                prev = cur
            emit_attn(n_wins - 1, *prev)
        else:
            for w in range(n_wins):
                qk_sb, v_sb = emit_qkv(w)
                emit_attn(w, qk_sb, v_sb)

    nc.compile()
    return nc


def host_prep(x, mask, w_qkv, w_out, bias_table, augv=None):
    """Build per-core input maps (numpy only)."""
    x = np.asarray(x, dtype=np.float32)
    mask = np.asarray(mask)
    w_qkv = np.asarray(w_qkv, dtype=np.float32)
    w_out = np.asarray(w_out, dtype=np.float32)
    bias_table = np.asarray(bias_table, dtype=np.float32)

    # x: (B, L, X, Y, W1, W2, C) -> windows (B,X,Y) x [C, N]
    xr = np.ascontiguousarray(x.transpose(0, 2, 3, 1, 4, 5, 6)).reshape(NWIN, N, DIM)
    xt = np.ascontiguousarray(xr.transpose(0, 2, 1)).reshape(NWIN, 2, 128, N).astype(np.float16)

    # mask: (B, X, Y, W1, W2, 1, L) -> (B,X,Y) x N with token order (l, w1, w2)
    m = np.ascontiguousarray(mask.transpose(0, 1, 2, 5, 6, 3, 4)).reshape(NWIN, N)
    maskadd = np.where(m == 0, np.float32(MASK_NEG), np.float32(0.0)).astype(np.float32)

    # exp(bias) transposed: ebT[h, j, i] = exp(bias[i, j, h])
    ri = _rel_pos_index(AGENT, WIN, WIN)
    bias = bias_table[ri]                       # (N, N, H) [i, j, h]
    ebT = np.exp(bias.transpose(2, 1, 0))       # (H, j, i)
    eb_host = np.ascontiguousarray(
        ebT.reshape(HEADS, NJC, JC, N).transpose(2, 0, 1, 3)
    ).astype(np.float16)                # (JC, H, NJC, N)

    wq = w_qkv.copy()
    wq[0:DIM] *= np.float32(SCALE)
    wqkvT = np.ascontiguousarray(wq.T).reshape(2, 128, 3 * DIM).astype(np.float16)
    if augv is None:
        augv = bool(BEST_CFG.get("augv"))
    if augv:
        # [4 head-pair tiles, 97 rows, DIM]: rows 0-31 head 2q, rows 64-95
        # head 2q+1, rows 32-63 & 96 zero (match the PV psum row layout)
        wt = np.asarray(w_out.T, dtype=np.float32)     # [c, e]
        woutT = np.zeros((4, 96, DIM), dtype=np.float16)
        for q in range(4):
            woutT[q, 0:32] = wt[64 * q:64 * q + 32].astype(np.float16)
            woutT[q, 64:96] = wt[64 * q + 32:64 * q + 64].astype(np.float16)
    else:
        woutT = np.ascontiguousarray(w_out.T).reshape(2, 128, DIM).astype(np.float16)

    in_maps = []
    for core in range(NCORES):
        ws = slice(WPC * core, WPC * (core + 1))
        mm = maskadd[ws].reshape(WPC, NJC, JC).transpose(2, 0, 1).reshape(JC, WPC * NJC)
        in_maps.append({
            "xt": np.ascontiguousarray(xt[ws]),
            "msk": np.ascontiguousarray(mm),
            "eb": eb_host,
            "wqkv": wqkvT,
            "wout": woutT,
        })
    return in_maps


def assemble_output(core_outs):
    """core_outs: list of [WPC, N, DIM] arrays -> full (B, L, X, Y, W1, W2, C)."""
    out = np.concatenate([np.asarray(o) for o in core_outs], axis=0).astype(np.float32)
    out = out.reshape(B, X, Y, AGENT, WIN, WIN, DIM)
    return np.ascontiguousarray(out.transpose(0, 3, 1, 2, 4, 5, 6)).astype(np.float32)


_NC_CACHE = {}


def _get_nc(n_wins=WPC):
    if n_wins not in _NC_CACHE:
        _NC_CACHE[n_wins] = build_graph(n_wins, **BEST_CFG)
    return _NC_CACHE[n_wins]


BEST_CFG = dict(wbufs=4, v_eng="dve", out16=True, sim_pairs=True, fo_eng="act", pipe='split2', one_dma=True, xt_pair=True, qk_eng='act', xt_pre=True)


def kernel(x, mask, w_qkv, w_out, bias_table):
    in_maps = host_prep(x, mask, w_qkv, w_out, bias_table)
    nc = _get_nc(WPC)
    res = run_bass_kernel_spmd(nc, in_maps, core_ids=list(range(NCORES)))
    core_outs = [res.results[i]["out"] for i in range(NCORES)]
    return assemble_output(core_outs)

